# revision 17
# baseline (speedup 1.0000x reference)
"""Trainium2 Bass kernel for nn_MoE_25005299597538 (moe_routing).

Strategy: data-parallel shard over tokens (N=8192 -> 1024 per core, 8 cores).
Each core runs: gate (softmax(scores/T), top-5 mask via 3x min-removal,
renormalize) + dense 3-layer MLP for all 8 experts + weighted combine.
No collectives needed; host concatenates the 8 output shards.

Layout trick: host feeds x pre-transposed (xT [D, NLOC]) so every matmul
contraction has K on partitions with zero on-device transposes:
  h1T[h, n]  = relu(W1 as lhsT  @ xT as rhs)      (+b1 via ACT bias)
  h2T[h2, n] = relu(W2 as lhsT  @ h1T as rhs)     (+b2 via ACT bias)
  out[n, o]  = h2T as lhsT @ W3 as rhs
  y[n, o]   += w[:, e] * out   (per-partition scalar; accum-DMA to DRAM)

The gate scores are computed with a 3-level bf16 split of x and gate_W
(6 cross-term matmuls -> exact products, fp32 accumulation) because the
PE's single-instruction f32/f32r matmuls are not accurate enough to
reproduce the fp32 reference's top-5 selection on near-tied scores.

MLP matmuls run in float32r (4x fp32 PE throughput, ~2e-4 rel precision).
The routing weights w are also written out; the host adds the (tiny)
`w @ b3` bias term and concatenates shards.
"""

import numpy as np

import concourse.bass as bass
import concourse.tile as tile
import concourse.mybir as mybir
from concourse import bacc

# Problem constants (hardcoded per contract; kernel.py must be self-contained).
N, D, H, O, E = 8192, 1024, 2048, 1024, 8
CORES = 8
NLOC = N // CORES  # 1024
TEMP = float(np.e)
N_ACTIVE = 5
EPS = 1e-8
BIG = 1e30

F32 = mybir.dt.float32


def build_nc(nloc=NLOC, d=D, h=H, o=O, n_experts=E, mm_dt=mybir.dt.float32r,
             n_active=N_ACTIVE, work_reps=1, diag="none"):
    """Build the per-core Bass program (SPMD: same program on all cores)."""
    P = 128
    DC = d // P            # contraction chunks for layer 1
    HC = h // P            # h chunks (layers 1/2 output, layers 2/3 contraction)
    NT = max(1, nloc // 512)   # free-dim tiles (512) for layers 1/2
    NW = min(512, nloc)
    QW = NW // P           # 128-token chunks per tile
    NQ = nloc // P         # 128-token chunks total
    OT = max(1, o // 512)  # output free-dim tiles for layer 3
    OW = min(512, o)
    KB = 2                 # W3 k-tiles batched per DMA
    n_drop = n_experts - n_active
    no_dma = diag == "nodma"

    nc = bacc.Bacc(None)

    BF16 = mybir.dt.bfloat16
    xT_ext = nc.dram_tensor("xT", [d, nloc], mm_dt, kind="ExternalInput")
    w1_ext = nc.dram_tensor("w1p", [n_experts, HC, P, DC, P], mm_dt, kind="ExternalInput")
    w2_ext = nc.dram_tensor("w2p", [n_experts, HC, P, HC, P], mm_dt, kind="ExternalInput")
    w3_ext = nc.dram_tensor("w3p", [n_experts, OT, HC, P, OW], mm_dt, kind="ExternalInput")
    # gate inputs as 3-level bf16 splits (x = xh+xm+xl exactly to ~2^-25):
    xts_ext = [nc.dram_tensor(f"xts{i}", [d, nloc], BF16, kind="ExternalInput")
               for i in range(3)]
    gws_ext = [nc.dram_tensor(f"gws{i}", [d, n_experts], BF16, kind="ExternalInput")
               for i in range(3)]
    gbs_ext = [nc.dram_tensor(f"gbs{i}", [1, n_experts], BF16, kind="ExternalInput")
               for i in range(3)]
    ones16_ext = nc.dram_tensor("ones16", [1, P], BF16, kind="ExternalInput")
    b1_ext = nc.dram_tensor("b1p", [P, n_experts, HC], F32, kind="ExternalInput")
    b2_ext = nc.dram_tensor("b2p", [P, n_experts, HC], F32, kind="ExternalInput")
    y_ext = nc.dram_tensor("y", [nloc, o], F32, kind="ExternalOutput")
    w_ext = nc.dram_tensor("wout", [nloc, n_experts], F32, kind="ExternalOutput")

    with tile.TileContext(nc) as tc:
        with tc.tile_pool(name="const", bufs=1) as cpool:
            # ---- resident loads -------------------------------------------
            xT_sb = cpool.tile([P, DC, nloc], mm_dt, tag="xT")
            nc.sync.dma_start(xT_sb[:], xT_ext.rearrange("(c p) n -> p c n", p=P))
            gws_sb = []
            for i in range(3):
                gw_sb = cpool.tile([P, DC, n_experts], BF16, tag=f"gws{i}",
                                   name=f"gws_sb{i}")
                nc.sync.dma_start(
                    gw_sb[:], gws_ext[i].rearrange("(c p) e -> p c e", p=P))
                gws_sb.append(gw_sb)
            gbs_sb = []
            for i in range(3):
                gb_sb = cpool.tile([1, n_experts], BF16, tag=f"gbs{i}",
                                   name=f"gbs_sb{i}")
                nc.sync.dma_start(gb_sb[:], gbs_ext[i][:])
                gbs_sb.append(gb_sb)
            ones16_sb = cpool.tile([1, P], BF16, tag="ones16")
            nc.sync.dma_start(ones16_sb[:], ones16_ext[:])
            b1_sb = cpool.tile([P, n_experts, HC], F32, tag="b1")
            nc.sync.dma_start(b1_sb[:], b1_ext[:])
            b2_sb = cpool.tile([P, n_experts, HC], F32, tag="b2")
            nc.sync.dma_start(b2_sb[:], b2_ext[:])
            w_all = cpool.tile([P, NQ, n_experts], F32, tag="w_all")

            # ---- gate: softmax(scores/T) -> top-5 -> renormalize ----------
            with (
                tc.tile_pool(name="gtmp", bufs=2) as gpool,
                tc.tile_pool(name="ps_g", bufs=2, space="PSUM") as gps,
            ):
                for q in range(NQ):
                    ps_g = gps.tile([P, n_experts], F32, tag="gate")
                    xq = []
                    for i in range(3):
                        xq_t = gpool.tile([P, DC, P], BF16, tag=f"xq{i}",
                                          name=f"xq{i}")
                        nc.sync.dma_start(
                            xq_t[:],
                            xts_ext[i].rearrange("(c p) n -> p c n", p=P)[
                                :, :, q * P:(q + 1) * P],
                        )
                        xq.append(xq_t)
                    # exact-products cross terms: hh, hm, mh, mm, hl, lh
                    first = True
                    for (a, b) in ((0, 0), (0, 1), (1, 0), (1, 1), (0, 2), (2, 0)):
                        for dc in range(DC):
                            nc.tensor.matmul(
                                ps_g[:], xq[a][:, dc, :], gws_sb[b][:, dc, :],
                                start=first, stop=False,
                            )
                            first = False
                    for i in range(3):
                        nc.tensor.matmul(
                            ps_g[:], ones16_sb[:], gbs_sb[i][:],
                            start=False, stop=(i == 2),
                        )
                    sc = gpool.tile([P, n_experts], F32, tag="sc")
                    nc.vector.tensor_copy(sc[:], ps_g[:])
                    mx = gpool.tile([P, 1], F32, tag="mx")
                    nc.vector.reduce_max(mx[:], sc[:], axis=mybir.AxisListType.X)
                    negb = gpool.tile([P, 1], F32, tag="negb")
                    nc.scalar.mul(negb[:], mx[:], -1.0 / TEMP)
                    pexp = gpool.tile([P, n_experts], F32, tag="pexp")
                    nc.scalar.activation(
                        pexp[:], sc[:], mybir.ActivationFunctionType.Exp,
                        bias=negb[:], scale=1.0 / TEMP,
                    )
                    ssum = gpool.tile([P, 1], F32, tag="ssum")
                    nc.vector.reduce_sum(ssum[:], pexp[:], axis=mybir.AxisListType.X)
                    rcp = gpool.tile([P, 1], F32, tag="rcp")
                    nc.vector.reciprocal(rcp[:], ssum[:])
                    probs = gpool.tile([P, n_experts], F32, tag="probs")
                    nc.vector.tensor_scalar_mul(probs[:], pexp[:], rcp[:])
                    work = gpool.tile([P, n_experts], F32, tag="work")
                    nc.vector.tensor_copy(work[:], probs[:])
                    for _ in range(n_drop):
                        mn = gpool.tile([P, 1], F32, tag="mn")
                        nc.vector.tensor_reduce(
                            mn[:], work[:], axis=mybir.AxisListType.X,
                            op=mybir.AluOpType.min,
                        )
                        hit = gpool.tile([P, n_experts], F32, tag="hit")
                        nc.vector.tensor_scalar(
                            hit[:], work[:], mn[:], BIG,
                            mybir.AluOpType.is_equal, mybir.AluOpType.mult,
                        )
                        nc.vector.tensor_add(work[:], work[:], hit[:])
                    keep = gpool.tile([P, n_experts], F32, tag="keep")
                    nc.vector.tensor_scalar(
                        keep[:], work[:], BIG / 2, None, mybir.AluOpType.is_lt,
                    )
                    wv = gpool.tile([P, n_experts], F32, tag="wv")
                    nc.vector.tensor_mul(wv[:], probs[:], keep[:])
                    s2 = gpool.tile([P, 1], F32, tag="s2")
                    nc.vector.reduce_sum(s2[:], wv[:], axis=mybir.AxisListType.X)
                    s2e = gpool.tile([P, 1], F32, tag="s2e")
                    nc.vector.tensor_scalar_add(s2e[:], s2[:], EPS)
                    r2 = gpool.tile([P, 1], F32, tag="r2")
                    nc.vector.reciprocal(r2[:], s2e[:])
                    nc.vector.tensor_scalar_mul(w_all[:, q, :], wv[:], r2[:])
                # write routing weights out (host applies the w @ b3 term)
                nc.sync.dma_start(
                    w_ext.rearrange("(q p) e -> p q e", p=P), w_all[:])

            # ---- experts ---------------------------------------------------
            with (
                tc.tile_pool(name="acts", bufs=2) as apool,
                tc.tile_pool(name="w1s", bufs=2) as w1pool,
                tc.tile_pool(name="w2s", bufs=2) as w2pool,
                tc.tile_pool(name="w3s", bufs=3) as w3pool,
                tc.tile_pool(name="yout", bufs=3) as ypool,
                tc.tile_pool(name="ps_mm", bufs=4, space="PSUM") as mmps,
                tc.tile_pool(name="ps_out", bufs=4, space="PSUM") as outps,
            ):
                if no_dma:
                    w1t_s = cpool.tile([P, DC, P], mm_dt, tag="w1t_s")
                    nc.sync.dma_start(w1t_s[:], w1_ext[0, 0])
                    w2t_s = cpool.tile([P, HC, P], mm_dt, tag="w2t_s")
                    nc.sync.dma_start(w2t_s[:], w2_ext[0, 0])
                    w3t_s = cpool.tile([P, KB, OW], mm_dt, tag="w3t_s")
                    nc.sync.dma_start(
                        w3t_s[:], w3_ext[0, 0, 0:KB].rearrange("k p w -> p k w"))

                for rep in range(work_reps):
                  for e in range(n_experts):
                    # layer 1: h1T[j] = relu(sum_dc W1t(j,dc).T @ xT(dc))
                    h1T = apool.tile([P, HC, nloc], mm_dt, tag="hact")
                    for j in range(HC):
                        if no_dma:
                            w1t = w1t_s
                        else:
                            w1t = w1pool.tile([P, DC, P], mm_dt, tag="w1t")
                            nc.sync.dma_start(w1t[:], w1_ext[e, j])
                        pss = [mmps.tile([P, NW], F32, tag="mm", name=f"psa{nt}")
                               for nt in range(NT)]
                        for dc in range(DC):
                            for nt in range(NT):
                                nc.tensor.matmul(
                                    pss[nt][:], w1t[:, dc, :],
                                    xT_sb[:, dc, nt * NW:(nt + 1) * NW],
                                    start=(dc == 0), stop=(dc == DC - 1),
                                )
                        for nt in range(NT):
                            nc.scalar.activation(
                                h1T[:, j, nt * NW:(nt + 1) * NW], pss[nt][:],
                                mybir.ActivationFunctionType.Relu,
                                bias=b1_sb[:, e, j:j + 1],
                            )
                    # layer 2: h2T[j2] = relu(sum_k W2t(j2,k).T @ h1T(k))
                    h2T = apool.tile([P, HC, nloc], mm_dt, tag="hact")
                    for j2 in range(HC):
                        if no_dma:
                            w2t = w2t_s
                        else:
                            w2t = w2pool.tile([P, HC, P], mm_dt, tag="w2t")
                            nc.sync.dma_start(w2t[:], w2_ext[e, j2])
                        pss = [mmps.tile([P, NW], F32, tag="mm", name=f"psb{nt}")
                               for nt in range(NT)]
                        for k in range(HC):
                            for nt in range(NT):
                                nc.tensor.matmul(
                                    pss[nt][:], w2t[:, k, :],
                                    h1T[:, k, nt * NW:(nt + 1) * NW],
                                    start=(k == 0), stop=(k == HC - 1),
                                )
                        for nt in range(NT):
                            nc.scalar.activation(
                                h2T[:, j2, nt * NW:(nt + 1) * NW], pss[nt][:],
                                mybir.ActivationFunctionType.Relu,
                                bias=b2_sb[:, e, j2:j2 + 1],
                            )
                    # layer 3 + combine, in groups of QW token-chunks
                    for qg in range(NT):
                        for ot in range(OT):
                            psO = [outps.tile([P, OW], F32, tag="out",
                                              name=f"psO{qq}")
                                   for qq in range(QW)]
                            for kb in range(HC // KB):
                                if no_dma:
                                    w3t = w3t_s
                                else:
                                    w3t = w3pool.tile([P, KB, OW], mm_dt,
                                                      tag="w3t")
                                    nc.sync.dma_start(
                                        w3t[:],
                                        w3_ext[e, ot, kb * KB:(kb + 1) * KB]
                                        .rearrange("k p w -> p k w"))
                                for ki in range(KB):
                                    k = kb * KB + ki
                                    for qq in range(QW):
                                        q = qg * QW + qq
                                        nc.tensor.matmul(
                                            psO[qq][:],
                                            h2T[:, k, q * P:(q + 1) * P],
                                            w3t[:, ki, :],
                                            start=(k == 0),
                                            stop=(k == HC - 1),
                                        )
                            for qq in range(QW):
                                q = qg * QW + qq
                                yt = ypool.tile([P, OW], F32, tag="yt")
                                nc.vector.tensor_scalar_mul(
                                    yt[:], psO[qq][:], w_all[:, q, e:e + 1])
                                nc.gpsimd.dma_start(
                                    y_ext[q * P:(q + 1) * P,
                                          ot * OW:(ot + 1) * OW],
                                    yt[:],
                                    accum_op=(mybir.AluOpType.bypass
                                              if (e == 0 and rep == 0)
                                              else mybir.AluOpType.add),
                                )
    nc.compile()
    return nc


# ---------------------------------------------------------------------------
# Host side: packing, PJRT runner (jit once, reusable), unshard.
# ---------------------------------------------------------------------------

def pack_inputs(x, gate_W, gate_b, W1, b1, W2, b2, W3, b3,
                nloc=NLOC, cores=CORES):
    """Shard + pre-tile the full inputs into per-core in_maps."""
    P = 128
    n_experts, d, h = W1.shape
    o = W3.shape[2]
    DC, HC = d // P, h // P
    OW = min(512, o)
    OT = max(1, o // OW)

    f32 = np.float32
    w1p = np.ascontiguousarray(
        W1.reshape(n_experts, DC, P, HC, P).transpose(0, 3, 2, 1, 4)).astype(f32)
    w2p = np.ascontiguousarray(
        W2.reshape(n_experts, HC, P, HC, P).transpose(0, 3, 2, 1, 4)).astype(f32)
    w3p = np.ascontiguousarray(
        W3.reshape(n_experts, HC, P, OT, OW).transpose(0, 3, 1, 2, 4)).astype(f32)
    b1p = np.ascontiguousarray(
        b1.reshape(n_experts, HC, P).transpose(2, 0, 1)).astype(f32)
    b2p = np.ascontiguousarray(
        b2.reshape(n_experts, HC, P).transpose(2, 0, 1)).astype(f32)

    import ml_dtypes
    bf16 = ml_dtypes.bfloat16

    def split3(a):
        a = a.astype(f32)
        hpart = a.astype(bf16)
        r = a - hpart.astype(f32)
        m = r.astype(bf16)
        low = (r - m.astype(f32)).astype(bf16)
        return [np.ascontiguousarray(t) for t in (hpart, m, low)]

    gws = split3(gate_W)
    gbs = split3(gate_b.reshape(1, n_experts))
    ones16 = np.ones((1, P), bf16)

    shared = dict(w1p=w1p, w2p=w2p, w3p=w3p,
                  b1p=b1p, b2p=b2p, ones16=ones16)
    for i in range(3):
        shared[f"gws{i}"] = gws[i]
        shared[f"gbs{i}"] = gbs[i]
    in_maps = []
    for c in range(cores):
        xT = np.ascontiguousarray(x[c * nloc:(c + 1) * nloc].T).astype(f32)
        xts = split3(xT)
        m = dict(xT=xT, **shared)
        for i in range(3):
            m[f"xts{i}"] = xts[i]
        in_maps.append(m)
    return in_maps


class SpmdRunner:
    """jit-once PJRT SPMD runner (mirrors bass2jax.run_bass_via_pjrt but
    reusable across calls so the NEFF compile is paid once)."""

    def __init__(self, nc, n_cores):
        import jax
        from jax.sharding import Mesh, PartitionSpec
        from jax.experimental.shard_map import shard_map
        from concourse import bass2jax as b2j

        b2j.install_neuronx_cc_hook()
        self.nc = nc
        self.n_cores = n_cores
        in_names, out_names, out_avals, zero_outs = [], [], [], []
        for alloc in nc.m.functions[0].allocations:
            if not isinstance(alloc, mybir.MemoryLocationSet):
                continue
            name = alloc.memorylocations[0].name
            if alloc.kind == "ExternalInput":
                if not (nc.partition_id_tensor
                        and name == nc.partition_id_tensor.name):
                    in_names.append(name)
            elif alloc.kind == "ExternalOutput":
                out_names.append(name)
                shape = tuple(alloc.tensor_shape)
                dtype = mybir.dt.np(alloc.dtype)
                out_avals.append(jax.core.ShapedArray(shape, dtype))
                zero_outs.append(np.zeros(shape, dtype))
        self.in_names, self.out_names = in_names, out_names
        self.out_avals, self.zero_outs = out_avals, zero_outs
        n_params, n_outs = len(in_names), len(out_names)
        self.n_params = n_params
        all_in_names = list(in_names) + list(out_names)
        partition_name = (nc.partition_id_tensor.name
                          if nc.partition_id_tensor else None)
        if partition_name is not None:
            all_in_names.append(partition_name)

        def _body(*args):
            operands = list(args)
            if partition_name is not None:
                operands.append(b2j.partition_id_tensor())
            outs = b2j._bass_exec_p.bind(
                *operands,
                out_avals=tuple(out_avals),
                in_names=tuple(all_in_names),
                out_names=tuple(out_names),
                lowering_input_output_aliases=(),
                sim_require_finite=True,
                sim_require_nnan=True,
                nc=nc,
            )
            return tuple(outs)

        devices = jax.devices()[:n_cores]
        self.mesh = Mesh(np.asarray(devices), ("core",))
        in_specs = (PartitionSpec("core"),) * (n_params + n_outs)
        out_specs = (PartitionSpec("core"),) * n_outs
        donate = tuple(range(n_params, n_params + n_outs))
        self.fn = jax.jit(
            shard_map(_body, mesh=self.mesh, in_specs=in_specs,
                      out_specs=out_specs, check_rep=False),
            donate_argnums=donate, keep_unused=True,
        )
        self.jax = jax

    def prep(self, in_maps):
        concat = [
            np.concatenate([np.asarray(m[n]) for m in in_maps], axis=0)
            for n in self.in_names
        ]
        return concat

    def zeros(self):
        return [np.zeros((self.n_cores * z.shape[0], *z.shape[1:]), z.dtype)
                for z in self.zero_outs]

    def __call__(self, concat_in, concat_zeros):
        out = self.fn(*concat_in, *concat_zeros)
        self.jax.block_until_ready(out)
        return out

    def time_pipelined(self, concat_in, k=33, reps=2):
        """Dispatch k executions back-to-back (async), block once: the device
        serializes them, so (T_k - T_1)/(k-1) ~= per-execution device time
        with dispatch overhead amortized."""
        import time as _time
        import jax
        from jax.sharding import NamedSharding, PartitionSpec
        sh = NamedSharding(self.mesh, PartitionSpec("core"))
        darrs = [jax.device_put(a, sh) for a in concat_in]
        jax.block_until_ready(darrs)

        def run_batch(n):
            zs = [[jax.device_put(z, sh) for z in self.zeros()]
                  for _ in range(n)]
            for z in zs:
                jax.block_until_ready(z)
            t0 = _time.perf_counter()
            outs = [self.fn(*darrs, *zs[i]) for i in range(n)]
            jax.block_until_ready(outs)
            return _time.perf_counter() - t0

        run_batch(2)  # warm
        t1 = min(run_batch(1) for _ in range(3))
        tk = min(run_batch(k) for _ in range(reps))
        per = (tk - t1) / (k - 1)
        return per, t1, tk

    def split_outs(self, out_arrs):
        res = []
        for c in range(self.n_cores):
            res.append({
                name: np.asarray(out_arrs[i]).reshape(
                    self.n_cores, *self.out_avals[i].shape)[c]
                for i, name in enumerate(self.out_names)
            })
        return res


_CACHE = {}


def _get_runner():
    if "runner" not in _CACHE:
        nc = build_nc()
        _CACHE["runner"] = SpmdRunner(nc, CORES)
    return _CACHE["runner"]


def kernel(**inputs):
    runner = _get_runner()
    b3 = np.asarray(inputs["b3"], dtype=np.float32)
    in_maps = pack_inputs(
        np.asarray(inputs["x"]), np.asarray(inputs["gate_W"]),
        np.asarray(inputs["gate_b"]), np.asarray(inputs["W1"]),
        np.asarray(inputs["b1"]), np.asarray(inputs["W2"]),
        np.asarray(inputs["b2"]), np.asarray(inputs["W3"]), b3,
    )
    out = runner(runner.prep(in_maps), runner.zeros())
    res = runner.split_outs(out)
    parts = []
    for c in range(CORES):
        y = res[c]["y"]
        w = res[c]["wout"]
        parts.append(y + w @ b3)
    return np.concatenate(parts, axis=0).astype(np.float32)


if __name__ == "__main__":
    rng = np.random.default_rng(0)
    x = rng.standard_normal((N, D), dtype=np.float32)
    print("building...")
    nc = build_nc()
    print("built ok")


# revision 19
# speedup vs baseline: 1.9977x; 1.9977x over previous
"""Trainium2 Bass kernel for nn_MoE_25005299597538 (moe_routing).

Strategy: data-parallel shard over tokens (N=8192 -> 1024 per core, 8 cores).
Each core runs: gate (softmax(scores/T), top-5 mask via 3x min-removal,
renormalize) + dense 3-layer MLP for all 8 experts + weighted combine.
No collectives needed; host concatenates the 8 output shards.

Layout trick: host feeds x pre-transposed (xT [D, NLOC]) so every matmul
contraction has K on partitions with zero on-device transposes:
  h1T[h, n]  = relu(W1 as lhsT  @ xT as rhs)      (+b1 via ACT bias)
  h2T[h2, n] = relu(W2 as lhsT  @ h1T as rhs)     (+b2 via ACT bias)
  out[n, o]  = h2T as lhsT @ W3 as rhs
  y[n, o]   += w[:, e] * out   (per-partition scalar; accum-DMA to DRAM)

The gate scores are computed with a 3-level bf16 split of x and gate_W
(6 cross-term matmuls -> exact products, fp32 accumulation) because the
PE's single-instruction f32/f32r matmuls are not accurate enough to
reproduce the fp32 reference's top-5 selection on near-tied scores.

MLP matmuls run in float32r (4x fp32 PE throughput, ~2e-4 rel precision).
The routing weights w are also written out; the host adds the (tiny)
`w @ b3` bias term and concatenates shards.
"""

import numpy as np

import concourse.bass as bass
import concourse.tile as tile
import concourse.mybir as mybir
from concourse import bacc

# Problem constants (hardcoded per contract; kernel.py must be self-contained).
N, D, H, O, E = 8192, 1024, 2048, 1024, 8
CORES = 8
NLOC = N // CORES  # 1024
TEMP = float(np.e)
N_ACTIVE = 5
EPS = 1e-8
BIG = 1e30

F32 = mybir.dt.float32


def build_nc(nloc=NLOC, d=D, h=H, o=O, n_experts=E, mm_dt=mybir.dt.float32r,
             n_active=N_ACTIVE, work_reps=1, diag="none"):
    """Build the per-core Bass program (SPMD: same program on all cores)."""
    P = 128
    DC = d // P            # contraction chunks for layer 1
    HC = h // P            # h chunks (layers 1/2 output, layers 2/3 contraction)
    NT = max(1, nloc // 512)   # free-dim tiles (512) for layers 1/2
    NW = min(512, nloc)
    QW = NW // P           # 128-token chunks per tile
    NQ = nloc // P         # 128-token chunks total
    OT = max(1, o // 512)  # output free-dim tiles for layer 3
    OW = min(512, o)
    KB = 2                 # W3 k-tiles batched per DMA
    n_drop = n_experts - n_active
    no_dma = diag == "nodma"

    nc = bacc.Bacc(None)

    BF16 = mybir.dt.bfloat16
    xT_ext = nc.dram_tensor("xT", [d, nloc], mm_dt, kind="ExternalInput")
    w1_ext = nc.dram_tensor("w1p", [n_experts, HC, P, DC, P], mm_dt, kind="ExternalInput")
    w2_ext = nc.dram_tensor("w2p", [n_experts, HC, P, HC, P], mm_dt, kind="ExternalInput")
    w3_ext = nc.dram_tensor("w3p", [n_experts, OT, HC, P, OW], mm_dt, kind="ExternalInput")
    # gate inputs as 3-level bf16 splits (x = xh+xm+xl exactly to ~2^-25):
    xts_ext = [nc.dram_tensor(f"xts{i}", [d, nloc], BF16, kind="ExternalInput")
               for i in range(3)]
    gws_ext = [nc.dram_tensor(f"gws{i}", [d, n_experts], BF16, kind="ExternalInput")
               for i in range(3)]
    gbs_ext = [nc.dram_tensor(f"gbs{i}", [1, n_experts], BF16, kind="ExternalInput")
               for i in range(3)]
    ones16_ext = nc.dram_tensor("ones16", [1, P], BF16, kind="ExternalInput")
    b1_ext = nc.dram_tensor("b1p", [P, n_experts, HC], F32, kind="ExternalInput")
    b2_ext = nc.dram_tensor("b2p", [P, n_experts, HC], F32, kind="ExternalInput")
    y_ext = nc.dram_tensor("y", [nloc, o], F32, kind="ExternalOutput")
    w_ext = nc.dram_tensor("wout", [nloc, n_experts], F32, kind="ExternalOutput")

    with tile.TileContext(nc) as tc:
        with tc.tile_pool(name="const", bufs=1) as cpool:
            # ---- resident loads -------------------------------------------
            xT_sb = cpool.tile([P, DC, nloc], mm_dt, tag="xT")
            # Activation's HWDGE queue: keeps the SP queue free for the gate's
            # small loads so the gate (and PE) starts sooner
            nc.scalar.dma_start(xT_sb[:], xT_ext.rearrange("(c p) n -> p c n", p=P))
            gws_sb = []
            for i in range(3):
                gw_sb = cpool.tile([P, DC, n_experts], BF16, tag=f"gws{i}",
                                   name=f"gws_sb{i}")
                nc.sync.dma_start(
                    gw_sb[:], gws_ext[i].rearrange("(c p) e -> p c e", p=P))
                gws_sb.append(gw_sb)
            gbs_sb = []
            for i in range(3):
                gb_sb = cpool.tile([1, n_experts], BF16, tag=f"gbs{i}",
                                   name=f"gbs_sb{i}")
                nc.sync.dma_start(gb_sb[:], gbs_ext[i][:])
                gbs_sb.append(gb_sb)
            ones16_sb = cpool.tile([1, P], BF16, tag="ones16")
            nc.sync.dma_start(ones16_sb[:], ones16_ext[:])
            b1_sb = cpool.tile([P, n_experts, HC], F32, tag="b1")
            nc.sync.dma_start(b1_sb[:], b1_ext[:])
            b2_sb = cpool.tile([P, n_experts, HC], F32, tag="b2")
            nc.sync.dma_start(b2_sb[:], b2_ext[:])
            w_all = cpool.tile([P, NQ, n_experts], F32, tag="w_all")

            # ---- gate: softmax(scores/T) -> top-5 -> renormalize ----------
            with (
                tc.tile_pool(name="gtmp", bufs=2) as gpool,
                tc.tile_pool(name="ps_g", bufs=2, space="PSUM") as gps,
            ):
                for q in range(NQ):
                    ps_g = gps.tile([P, n_experts], F32, tag="gate")
                    xq = []
                    for i in range(3):
                        xq_t = gpool.tile([P, DC, P], BF16, tag=f"xq{i}",
                                          name=f"xq{i}")
                        nc.sync.dma_start(
                            xq_t[:],
                            xts_ext[i].rearrange("(c p) n -> p c n", p=P)[
                                :, :, q * P:(q + 1) * P],
                        )
                        xq.append(xq_t)
                    # exact-products cross terms: hh, hm, mh, mm, hl, lh
                    first = True
                    for (a, b) in ((0, 0), (0, 1), (1, 0), (1, 1), (0, 2), (2, 0)):
                        for dc in range(DC):
                            nc.tensor.matmul(
                                ps_g[:], xq[a][:, dc, :], gws_sb[b][:, dc, :],
                                start=first, stop=False,
                            )
                            first = False
                    for i in range(3):
                        nc.tensor.matmul(
                            ps_g[:], ones16_sb[:], gbs_sb[i][:],
                            start=False, stop=(i == 2),
                        )
                    sc = gpool.tile([P, n_experts], F32, tag="sc")
                    nc.vector.tensor_copy(sc[:], ps_g[:])
                    mx = gpool.tile([P, 1], F32, tag="mx")
                    nc.vector.reduce_max(mx[:], sc[:], axis=mybir.AxisListType.X)
                    negb = gpool.tile([P, 1], F32, tag="negb")
                    nc.scalar.mul(negb[:], mx[:], -1.0 / TEMP)
                    pexp = gpool.tile([P, n_experts], F32, tag="pexp")
                    nc.scalar.activation(
                        pexp[:], sc[:], mybir.ActivationFunctionType.Exp,
                        bias=negb[:], scale=1.0 / TEMP,
                    )
                    ssum = gpool.tile([P, 1], F32, tag="ssum")
                    nc.vector.reduce_sum(ssum[:], pexp[:], axis=mybir.AxisListType.X)
                    rcp = gpool.tile([P, 1], F32, tag="rcp")
                    nc.vector.reciprocal(rcp[:], ssum[:])
                    probs = gpool.tile([P, n_experts], F32, tag="probs")
                    nc.vector.tensor_scalar_mul(probs[:], pexp[:], rcp[:])
                    work = gpool.tile([P, n_experts], F32, tag="work")
                    nc.vector.tensor_copy(work[:], probs[:])
                    for _ in range(n_drop):
                        mn = gpool.tile([P, 1], F32, tag="mn")
                        nc.vector.tensor_reduce(
                            mn[:], work[:], axis=mybir.AxisListType.X,
                            op=mybir.AluOpType.min,
                        )
                        hit = gpool.tile([P, n_experts], F32, tag="hit")
                        nc.vector.tensor_scalar(
                            hit[:], work[:], mn[:], BIG,
                            mybir.AluOpType.is_equal, mybir.AluOpType.mult,
                        )
                        nc.vector.tensor_add(work[:], work[:], hit[:])
                    keep = gpool.tile([P, n_experts], F32, tag="keep")
                    nc.vector.tensor_scalar(
                        keep[:], work[:], BIG / 2, None, mybir.AluOpType.is_lt,
                    )
                    wv = gpool.tile([P, n_experts], F32, tag="wv")
                    nc.vector.tensor_mul(wv[:], probs[:], keep[:])
                    s2 = gpool.tile([P, 1], F32, tag="s2")
                    nc.vector.reduce_sum(s2[:], wv[:], axis=mybir.AxisListType.X)
                    s2e = gpool.tile([P, 1], F32, tag="s2e")
                    nc.vector.tensor_scalar_add(s2e[:], s2[:], EPS)
                    r2 = gpool.tile([P, 1], F32, tag="r2")
                    nc.vector.reciprocal(r2[:], s2e[:])
                    nc.vector.tensor_scalar_mul(w_all[:, q, :], wv[:], r2[:])
                # write routing weights out (host applies the w @ b3 term)
                nc.sync.dma_start(
                    w_ext.rearrange("(q p) e -> p q e", p=P), w_all[:])

            # ---- experts ---------------------------------------------------
            with (
                tc.tile_pool(name="acts", bufs=2) as apool,
                tc.tile_pool(name="w1s", bufs=2) as w1pool,
                tc.tile_pool(name="w2s", bufs=2) as w2pool,
                tc.tile_pool(name="w3s", bufs=3) as w3pool,
                tc.tile_pool(name="yout", bufs=3) as ypool,
                tc.tile_pool(name="ps_mm", bufs=4, space="PSUM") as mmps,
                tc.tile_pool(name="ps_out", bufs=4, space="PSUM") as outps,
            ):
                if no_dma:
                    w1t_s = cpool.tile([P, DC, P], mm_dt, tag="w1t_s")
                    nc.sync.dma_start(w1t_s[:], w1_ext[0, 0])
                    w2t_s = cpool.tile([P, HC, P], mm_dt, tag="w2t_s")
                    nc.sync.dma_start(w2t_s[:], w2_ext[0, 0])
                    w3t_s = cpool.tile([P, KB, OW], mm_dt, tag="w3t_s")
                    nc.sync.dma_start(
                        w3t_s[:], w3_ext[0, 0, 0:KB].rearrange("k p w -> p k w"))

                for rep in range(work_reps):
                  for e in range(n_experts):
                    # layer 1: h1T[j] = relu(sum_dc W1t(j,dc).T @ xT(dc))
                    h1T = apool.tile([P, HC, nloc], mm_dt, tag="hact")
                    for j in range(HC):
                        if no_dma:
                            w1t = w1t_s
                        else:
                            w1t = w1pool.tile([P, DC, P], mm_dt, tag="w1t")
                            nc.sync.dma_start(w1t[:], w1_ext[e, j])
                        pss = [mmps.tile([P, NW], F32, tag="mm", name=f"psa{nt}")
                               for nt in range(NT)]
                        for dc in range(DC):
                            for nt in range(NT):
                                nc.tensor.matmul(
                                    pss[nt][:], w1t[:, dc, :],
                                    xT_sb[:, dc, nt * NW:(nt + 1) * NW],
                                    start=(dc == 0), stop=(dc == DC - 1),
                                )
                        for nt in range(NT):
                            nc.scalar.activation(
                                h1T[:, j, nt * NW:(nt + 1) * NW], pss[nt][:],
                                mybir.ActivationFunctionType.Relu,
                                bias=b1_sb[:, e, j:j + 1],
                            )
                    # layer 2: h2T[j2] = relu(sum_k W2t(j2,k).T @ h1T(k))
                    h2T = apool.tile([P, HC, nloc], mm_dt, tag="hact")
                    for j2 in range(HC):
                        if no_dma:
                            w2t = w2t_s
                        else:
                            w2t = w2pool.tile([P, HC, P], mm_dt, tag="w2t")
                            # W2 is the largest stream (16MB/expert): issue on
                            # the Activation HWDGE queue, in parallel with the
                            # W1/W3 streams on the SP queue
                            nc.scalar.dma_start(w2t[:], w2_ext[e, j2])
                        pss = [mmps.tile([P, NW], F32, tag="mm", name=f"psb{nt}")
                               for nt in range(NT)]
                        for k in range(HC):
                            for nt in range(NT):
                                nc.tensor.matmul(
                                    pss[nt][:], w2t[:, k, :],
                                    h1T[:, k, nt * NW:(nt + 1) * NW],
                                    start=(k == 0), stop=(k == HC - 1),
                                )
                        for nt in range(NT):
                            nc.scalar.activation(
                                h2T[:, j2, nt * NW:(nt + 1) * NW], pss[nt][:],
                                mybir.ActivationFunctionType.Relu,
                                bias=b2_sb[:, e, j2:j2 + 1],
                            )
                    # layer 3 + combine, in groups of QW token-chunks
                    for qg in range(NT):
                        for ot in range(OT):
                            psO = [outps.tile([P, OW], F32, tag="out",
                                              name=f"psO{qq}")
                                   for qq in range(QW)]
                            for kb in range(HC // KB):
                                if no_dma:
                                    w3t = w3t_s
                                else:
                                    w3t = w3pool.tile([P, KB, OW], mm_dt,
                                                      tag="w3t")
                                    nc.sync.dma_start(
                                        w3t[:],
                                        w3_ext[e, ot, kb * KB:(kb + 1) * KB]
                                        .rearrange("k p w -> p k w"))
                                for ki in range(KB):
                                    k = kb * KB + ki
                                    for qq in range(QW):
                                        q = qg * QW + qq
                                        nc.tensor.matmul(
                                            psO[qq][:],
                                            h2T[:, k, q * P:(q + 1) * P],
                                            w3t[:, ki, :],
                                            start=(k == 0),
                                            stop=(k == HC - 1),
                                        )
                            for qq in range(QW):
                                q = qg * QW + qq
                                yt = ypool.tile([P, OW], F32, tag="yt")
                                nc.vector.tensor_scalar_mul(
                                    yt[:], psO[qq][:], w_all[:, q, e:e + 1])
                                nc.gpsimd.dma_start(
                                    y_ext[q * P:(q + 1) * P,
                                          ot * OW:(ot + 1) * OW],
                                    yt[:],
                                    accum_op=(mybir.AluOpType.bypass
                                              if (e == 0 and rep == 0)
                                              else mybir.AluOpType.add),
                                )
    nc.compile()
    return nc


# ---------------------------------------------------------------------------
# Host side: packing, PJRT runner (jit once, reusable), unshard.
# ---------------------------------------------------------------------------

def pack_inputs(x, gate_W, gate_b, W1, b1, W2, b2, W3, b3,
                nloc=NLOC, cores=CORES):
    """Shard + pre-tile the full inputs into per-core in_maps."""
    P = 128
    n_experts, d, h = W1.shape
    o = W3.shape[2]
    DC, HC = d // P, h // P
    OW = min(512, o)
    OT = max(1, o // OW)

    f32 = np.float32
    w1p = np.ascontiguousarray(
        W1.reshape(n_experts, DC, P, HC, P).transpose(0, 3, 2, 1, 4)).astype(f32)
    w2p = np.ascontiguousarray(
        W2.reshape(n_experts, HC, P, HC, P).transpose(0, 3, 2, 1, 4)).astype(f32)
    w3p = np.ascontiguousarray(
        W3.reshape(n_experts, HC, P, OT, OW).transpose(0, 3, 1, 2, 4)).astype(f32)
    b1p = np.ascontiguousarray(
        b1.reshape(n_experts, HC, P).transpose(2, 0, 1)).astype(f32)
    b2p = np.ascontiguousarray(
        b2.reshape(n_experts, HC, P).transpose(2, 0, 1)).astype(f32)

    import ml_dtypes
    bf16 = ml_dtypes.bfloat16

    def split3(a):
        a = a.astype(f32)
        hpart = a.astype(bf16)
        r = a - hpart.astype(f32)
        m = r.astype(bf16)
        low = (r - m.astype(f32)).astype(bf16)
        return [np.ascontiguousarray(t) for t in (hpart, m, low)]

    gws = split3(gate_W)
    gbs = split3(gate_b.reshape(1, n_experts))
    ones16 = np.ones((1, P), bf16)

    shared = dict(w1p=w1p, w2p=w2p, w3p=w3p,
                  b1p=b1p, b2p=b2p, ones16=ones16)
    for i in range(3):
        shared[f"gws{i}"] = gws[i]
        shared[f"gbs{i}"] = gbs[i]
    in_maps = []
    for c in range(cores):
        xT = np.ascontiguousarray(x[c * nloc:(c + 1) * nloc].T).astype(f32)
        xts = split3(xT)
        m = dict(xT=xT, **shared)
        for i in range(3):
            m[f"xts{i}"] = xts[i]
        in_maps.append(m)
    return in_maps


class SpmdRunner:
    """jit-once PJRT SPMD runner (mirrors bass2jax.run_bass_via_pjrt but
    reusable across calls so the NEFF compile is paid once)."""

    def __init__(self, nc, n_cores):
        import jax
        from jax.sharding import Mesh, PartitionSpec
        from jax.experimental.shard_map import shard_map
        from concourse import bass2jax as b2j

        b2j.install_neuronx_cc_hook()
        self.nc = nc
        self.n_cores = n_cores
        in_names, out_names, out_avals, zero_outs = [], [], [], []
        for alloc in nc.m.functions[0].allocations:
            if not isinstance(alloc, mybir.MemoryLocationSet):
                continue
            name = alloc.memorylocations[0].name
            if alloc.kind == "ExternalInput":
                if not (nc.partition_id_tensor
                        and name == nc.partition_id_tensor.name):
                    in_names.append(name)
            elif alloc.kind == "ExternalOutput":
                out_names.append(name)
                shape = tuple(alloc.tensor_shape)
                dtype = mybir.dt.np(alloc.dtype)
                out_avals.append(jax.core.ShapedArray(shape, dtype))
                zero_outs.append(np.zeros(shape, dtype))
        self.in_names, self.out_names = in_names, out_names
        self.out_avals, self.zero_outs = out_avals, zero_outs
        n_params, n_outs = len(in_names), len(out_names)
        self.n_params = n_params
        all_in_names = list(in_names) + list(out_names)
        partition_name = (nc.partition_id_tensor.name
                          if nc.partition_id_tensor else None)
        if partition_name is not None:
            all_in_names.append(partition_name)

        def _body(*args):
            operands = list(args)
            if partition_name is not None:
                operands.append(b2j.partition_id_tensor())
            outs = b2j._bass_exec_p.bind(
                *operands,
                out_avals=tuple(out_avals),
                in_names=tuple(all_in_names),
                out_names=tuple(out_names),
                lowering_input_output_aliases=(),
                sim_require_finite=True,
                sim_require_nnan=True,
                nc=nc,
            )
            return tuple(outs)

        devices = jax.devices()[:n_cores]
        self.mesh = Mesh(np.asarray(devices), ("core",))
        in_specs = (PartitionSpec("core"),) * (n_params + n_outs)
        out_specs = (PartitionSpec("core"),) * n_outs
        donate = tuple(range(n_params, n_params + n_outs))
        self.fn = jax.jit(
            shard_map(_body, mesh=self.mesh, in_specs=in_specs,
                      out_specs=out_specs, check_rep=False),
            donate_argnums=donate, keep_unused=True,
        )
        self.jax = jax

    def prep(self, in_maps):
        concat = [
            np.concatenate([np.asarray(m[n]) for m in in_maps], axis=0)
            for n in self.in_names
        ]
        return concat

    def zeros(self):
        return [np.zeros((self.n_cores * z.shape[0], *z.shape[1:]), z.dtype)
                for z in self.zero_outs]

    def __call__(self, concat_in, concat_zeros):
        out = self.fn(*concat_in, *concat_zeros)
        self.jax.block_until_ready(out)
        return out

    def time_pipelined(self, concat_in, k=33, reps=2):
        """Dispatch k executions back-to-back (async), block once: the device
        serializes them, so (T_k - T_1)/(k-1) ~= per-execution device time
        with dispatch overhead amortized."""
        import time as _time
        import jax
        from jax.sharding import NamedSharding, PartitionSpec
        sh = NamedSharding(self.mesh, PartitionSpec("core"))
        darrs = [jax.device_put(a, sh) for a in concat_in]
        jax.block_until_ready(darrs)

        def run_batch(n):
            zs = [[jax.device_put(z, sh) for z in self.zeros()]
                  for _ in range(n)]
            for z in zs:
                jax.block_until_ready(z)
            t0 = _time.perf_counter()
            outs = [self.fn(*darrs, *zs[i]) for i in range(n)]
            jax.block_until_ready(outs)
            return _time.perf_counter() - t0

        run_batch(2)  # warm
        t1 = min(run_batch(1) for _ in range(3))
        tk = min(run_batch(k) for _ in range(reps))
        per = (tk - t1) / (k - 1)
        return per, t1, tk

    def split_outs(self, out_arrs):
        res = []
        for c in range(self.n_cores):
            res.append({
                name: np.asarray(out_arrs[i]).reshape(
                    self.n_cores, *self.out_avals[i].shape)[c]
                for i, name in enumerate(self.out_names)
            })
        return res


_CACHE = {}


def _get_runner():
    if "runner" not in _CACHE:
        nc = build_nc()
        _CACHE["runner"] = SpmdRunner(nc, CORES)
    return _CACHE["runner"]


def kernel(**inputs):
    runner = _get_runner()
    b3 = np.asarray(inputs["b3"], dtype=np.float32)
    in_maps = pack_inputs(
        np.asarray(inputs["x"]), np.asarray(inputs["gate_W"]),
        np.asarray(inputs["gate_b"]), np.asarray(inputs["W1"]),
        np.asarray(inputs["b1"]), np.asarray(inputs["W2"]),
        np.asarray(inputs["b2"]), np.asarray(inputs["W3"]), b3,
    )
    out = runner(runner.prep(in_maps), runner.zeros())
    res = runner.split_outs(out)
    parts = []
    for c in range(CORES):
        y = res[c]["y"]
        w = res[c]["wout"]
        parts.append(y + w @ b3)
    return np.concatenate(parts, axis=0).astype(np.float32)


if __name__ == "__main__":
    rng = np.random.default_rng(0)
    x = rng.standard_normal((N, D), dtype=np.float32)
    print("building...")
    nc = build_nc()
    print("built ok")
